# revision 16
# baseline (speedup 1.0000x reference)
"""Trainium2 Bass kernel for ChunkMessagePassing (gnn_message_passing).

Problem: B=2, N=4096, D=512, 3 rounds of causal windowed (W=8) message
passing. Per round:
    A = h @ w1_top + b1 ; Bv = h @ w1_bot          (first MLP layer, factored)
    S[i] = sum_{k=0..8, valid} gelu(A[i] + Bv[i-k])
    agg[i] = (S[i]/cnt[i]) @ w2 + b2               (sum commutes with linear)
    new_h = h + MLP_upd([h, agg]) ; h = LN(new_h)

Sharding: 8 cores = B(2) x N-quarters(4). Each core gets 1024 tokens plus a
24-token left halo (3 rounds x window 8), computed redundantly. Zero
cross-core communication. Cores at a sequence start get a zero pad instead
of a halo plus a data-driven edge fixup (invalid window taps excluded,
window count < 9) so all 8 cores run one SPMD program.

Layout: D on partitions (4 tiles of 128), tokens on the free axis.
Matmuls in fp32r (1 cyc/row at free>=256, ~1e-4 rel err). Window stage in
bf16 for DVE 2x mode; shifted reads stay 4B-aligned via an odd-shifted Bv
copy. LN stats via ones-matmul over partitions, broadcast back via K=1
matmul. The round body is chunk-pipelined: every stage runs per 262-token
chunk so PE matmul work for chunk c+1 overlaps the DVE/ACT window work of
chunk c.
"""

import numpy as np
import ml_dtypes

import concourse.bacc as bacc
import concourse.mybir as mybir
from concourse.tile import TileContext
from concourse.bass_utils import run_bass_kernel_spmd

f32 = mybir.dt.float32
f32r = mybir.dt.float32r
bf16 = mybir.dt.bfloat16
AF = mybir.ActivationFunctionType
ALU = mybir.AluOpType

B, N, D = 2, 4096, 512
N_ROUNDS = 3
W = 8
W9 = W + 1
NCORES = 8
NLOC = N // 4            # tokens owned per core
HALO = N_ROUNDS * W      # 24
T = NLOC + HALO          # 1048 local tokens incl. halo
DT = 4                   # number of 128-partition d tiles
P = 128
MARG = 8                 # zero margin on the left of Bv buffers
CN = 352                 # max chunk width (>=256 keeps fp32r at 1 cyc/row)
CHUNKS = [(0, 352), (352, 352), (704, 344)]
EPS = 1e-5


def build_nc():
    nc = bacc.Bacc("TRN2")

    # ---- DRAM I/O (per-core data supplied via in_maps) ----
    h_in = nc.dram_tensor("h_in", [DT, P, T], f32r, kind="ExternalInput")
    w1t_d = nc.dram_tensor("w1t", [DT, P, D], f32r, kind="ExternalInput")
    w1b_d = nc.dram_tensor("w1b", [DT, P, D], f32r, kind="ExternalInput")
    u1t_d = nc.dram_tensor("u1t", [DT, P, D], f32r, kind="ExternalInput")
    u1b_d = nc.dram_tensor("u1b", [DT, P, D], bf16, kind="ExternalInput")
    w2s_d = nc.dram_tensor("w2s", [DT, P, D], bf16, kind="ExternalInput")
    u2_d = nc.dram_tensor("u2", [DT, P, D], bf16, kind="ExternalInput")
    b1_d = nc.dram_tensor("b1", [P, DT], f32, kind="ExternalInput")
    b2_d = nc.dram_tensor("b2", [P, DT], f32, kind="ExternalInput")
    ub1_d = nc.dram_tensor("ub1", [P, DT], f32, kind="ExternalInput")
    ub2_d = nc.dram_tensor("ub2", [P, DT], f32, kind="ExternalInput")
    lng_d = nc.dram_tensor("lng", [P, DT], f32, kind="ExternalInput")
    lnb_d = nc.dram_tensor("lnb", [P, DT], f32, kind="ExternalInput")
    iden_d = nc.dram_tensor("iden", [P, P], f32r, kind="ExternalInput")
    ea_d = nc.dram_tensor("edge_a", [P, W], bf16, kind="ExternalInput")
    es_d = nc.dram_tensor("edge_s", [P, W], bf16, kind="ExternalInput")
    hm_d = nc.dram_tensor("hmask", [P, HALO], f32, kind="ExternalInput")
    out_d = nc.dram_tensor("out", [DT, P, NLOC], f32, kind="ExternalOutput")

    with nc.allow_low_precision("bf16/f32r compute validated against reference"), \
            TileContext(nc) as tc:
        with (
            tc.tile_pool(name="const", bufs=1) as cp,
            tc.tile_pool(name="acts", bufs=1) as ap,
            tc.tile_pool(name="wsc", bufs=3) as wp,
            tc.tile_pool(name="psab", bufs=3, space="PSUM") as psab,
            tc.tile_pool(name="ps", bufs=3, space="PSUM") as ps,
            tc.tile_pool(name="psr", bufs=2, space="PSUM") as psr,
        ):
            # ---- constants into SBUF ----
            w1t = cp.tile([P, DT * D], f32r, tag="w1t")
            w1b = cp.tile([P, DT * D], f32r, tag="w1b")
            u1t = cp.tile([P, DT * D], f32r, tag="u1t")
            u1b = cp.tile([P, DT * D], bf16, tag="u1b")
            w2s = cp.tile([P, DT * D], bf16, tag="w2s")
            u2 = cp.tile([P, DT * D], bf16, tag="u2")
            iden = cp.tile([P, P], f32r, tag="iden")
            for t_sb, t_d in ((w1t, w1t_d), (w1b, w1b_d), (u1t, u1t_d),
                              (u1b, u1b_d), (w2s, w2s_d), (u2, u2_d)):
                nc.sync.dma_start(
                    out=t_sb[:].rearrange("p (k d) -> p k d", k=DT),
                    in_=t_d.rearrange("k p d -> p k d"))
            nc.sync.dma_start(out=iden[:], in_=iden_d[:])
            b1 = cp.tile([P, DT], f32, tag="b1")
            b2 = cp.tile([P, DT], f32, tag="b2")
            ub1 = cp.tile([P, DT], f32, tag="ub1")
            ub2 = cp.tile([P, DT], f32, tag="ub2")
            lng = cp.tile([P, DT], f32, tag="lng")
            lnb = cp.tile([P, DT], f32, tag="lnb")
            edge_a = cp.tile([P, W], bf16, tag="edge_a")
            edge_s = cp.tile([P, W], bf16, tag="edge_s")
            hmask = cp.tile([P, HALO], f32, tag="hmask")
            for t_sb, t_d in ((b1, b1_d), (b2, b2_d), (ub1, ub1_d), (ub2, ub2_d),
                              (lng, lng_d), (lnb, lnb_d), (edge_a, ea_d),
                              (edge_s, es_d), (hmask, hm_d)):
                nc.scalar.dma_start(out=t_sb[:], in_=t_d[:])

            ones_col = cp.tile([P, 1], f32r, tag="ones_col")   # stats lhsT
            ones_row = cp.tile([1, P], f32r, tag="ones_row")   # bcast lhsT
            ones_f = cp.tile([P, 1], f32, tag="ones_f")
            nc.vector.memset(ones_f[:], 1.0)
            nc.vector.tensor_copy(ones_col[:], ones_f[:])
            nc.vector.tensor_copy(ones_row[:], ones_f[:1, :].to_broadcast([1, P]))
            czero = cp.tile([P, 1], f32, tag="czero")
            ceps = cp.tile([P, 1], f32, tag="ceps")
            nc.vector.memset(czero[:], 0.0)
            nc.vector.memset(ceps[:], EPS)
            nc.const_aps.aps[(f32, 0.0)] = czero[:]
            nc.const_aps.aps[(f32, EPS)] = ceps[:]

            # ---- activations (persistent, reused across rounds) ----
            h0 = ap.tile([P, DT * T], f32r, tag="h0")
            h1 = ap.tile([P, DT * T], f32r, tag="h1")
            A = ap.tile([P, DT * T], bf16, tag="A")
            BVW = MARG + T + 2
            Bv = ap.tile([P, DT * BVW], bf16, tag="Bv")
            Bvo = ap.tile([P, DT * BVW], bf16, tag="Bvo")
            S = ap.tile([P, DT * T], bf16, tag="S")
            agg = ap.tile([P, DT * T], bf16, tag="agg")
            x2 = ap.tile([P, DT * T], f32r, tag="x2")
            rowAB = ap.tile([1, 2 * T], f32r, tag="rowAB")   # [-mu*rstd | rstd]
            rowCD = ap.tile([1, 2 * T], f32, tag="rowCD")
            ga8 = ap.tile([P, W], bf16, tag="ga8")
            xn = x2        # aliases: x2[*,c] dead (stats read) before xn[*,c]
            G = A          # G written after A's last read each round

            for dt in range(DT):
                nc.vector.memset(Bv[:, dt * BVW: dt * BVW + MARG], 0.0)
                nc.vector.memset(Bvo[:, dt * BVW: dt * BVW + MARG + 1], 0.0)

            # round-1 input, chunked so stage-1 starts on the first chunk;
            # separate queue (gpsimd) so it overlaps the weight DMAs
            for (c0, cn) in CHUNKS:
                for dt in range(DT):
                    nc.gpsimd.dma_start(out=h0[:, dt * T + c0: dt * T + c0 + cn],
                                        in_=h_in[dt, :, c0: c0 + cn])

            def hsl(h, dt, c0, n):
                return h[:, dt * T + c0: dt * T + c0 + n]

            def wtile(w, kt, dt):
                return w[:, kt * D + dt * P: kt * D + dt * P + P]

            hbufs = [h0, h1]
            for r in range(N_ROUNDS):
                hin = hbufs[r % 2]
                hout = hbufs[(r + 1) % 2]

                for ci, (c0, cn) in enumerate(CHUNKS):
                    # ---- stage 1: A / Bv matmuls for this chunk
                    for dt in range(DT):
                        pa = psab.tile([P, 512], f32, tag="pab")
                        for kt in range(DT):
                            nc.tensor.matmul(pa[:, :cn], wtile(w1t, kt, dt),
                                             hsl(hin, kt, c0, cn),
                                             start=(kt == 0), stop=(kt == DT - 1))
                        nc.scalar.activation(A[:, dt * T + c0: dt * T + c0 + cn],
                                             pa[:, :cn], AF.Copy)
                        pb = psab.tile([P, 512], f32, tag="pab")
                        for kt in range(DT):
                            nc.tensor.matmul(pb[:, :cn], wtile(w1b, kt, dt),
                                             hsl(hin, kt, c0, cn),
                                             start=(kt == 0), stop=(kt == DT - 1))
                        base = dt * BVW + MARG + c0
                        nc.scalar.activation(Bv[:, base: base + cn], pb[:, :cn],
                                             AF.Copy)
                        nc.vector.tensor_copy(Bvo[:, base + 1: base + 1 + cn],
                                              Bv[:, base: base + cn])

                    # ---- stage 2: windowed gelu-sum -> S (this chunk)
                    for dt in range(DT):
                        tmp = wp.tile([P, W9 * CN], bf16, tag="tmp")
                        g = wp.tile([P, W9 * CN], bf16, tag="g")
                        a_sl = A[:, dt * T + c0: dt * T + c0 + cn]
                        for k in range(W9):
                            if k % 2 == 0:
                                bsl = Bv[:, dt * BVW + MARG - k + c0:
                                         dt * BVW + MARG - k + c0 + cn]
                            else:
                                bsl = Bvo[:, dt * BVW + MARG + 1 - k + c0:
                                          dt * BVW + MARG + 1 - k + c0 + cn]
                            nc.vector.tensor_tensor(
                                tmp[:, k * cn:(k + 1) * cn], a_sl, bsl, ALU.add)
                        nc.scalar.activation(g[:, : W9 * cn], tmp[:, : W9 * cn],
                                             AF.Gelu, bias=b1[:, dt: dt + 1])
                        nc.vector.tensor_tensor(tmp[:, 0: 4 * cn], g[:, 0: 4 * cn],
                                                g[:, 4 * cn: 8 * cn], ALU.add)
                        nc.vector.tensor_tensor(tmp[:, 0: 2 * cn], tmp[:, 0: 2 * cn],
                                                tmp[:, 2 * cn: 4 * cn], ALU.add)
                        nc.gpsimd.tensor_tensor(tmp[:, 0: cn], tmp[:, 0: cn],
                                                tmp[:, cn: 2 * cn], ALU.add)
                        nc.gpsimd.tensor_tensor(
                            S[:, dt * T + c0: dt * T + c0 + cn],
                            tmp[:, 0: cn], g[:, 8 * cn: 9 * cn], ALU.add)

                    # ---- edge fixup (chunk 0 only; no-op off sequence starts)
                    if ci == 0:
                        for dt in range(DT):
                            sle = S[:, dt * T + HALO: dt * T + HALO + W]
                            nc.scalar.activation(
                                ga8[:], A[:, dt * T + HALO: dt * T + HALO + W],
                                AF.Gelu, bias=b1[:, dt: dt + 1])
                            nc.vector.tensor_tensor(ga8[:], ga8[:], edge_a[:],
                                                    ALU.mult)
                            nc.vector.tensor_tensor(sle, sle, ga8[:], ALU.subtract)
                            nc.vector.tensor_tensor(sle, sle, edge_s[:], ALU.mult)

                    # ---- stage 3: agg = S @ w2s + b2
                    for dt in range(DT):
                        pg = ps.tile([P, 512], f32, tag="pmm")
                        for kt in range(DT):
                            nc.tensor.matmul(pg[:, :cn], wtile(w2s, kt, dt),
                                             S[:, kt * T + c0: kt * T + c0 + cn],
                                             start=(kt == 0), stop=(kt == DT - 1))
                        nc.scalar.activation(agg[:, dt * T + c0: dt * T + c0 + cn],
                                             pg[:, :cn], AF.Identity,
                                             bias=b2[:, dt: dt + 1])

                    # ---- stage 4: U = u1t.T@h + u1b.T@agg ; G = gelu(U+ub1)
                    for dt in range(DT):
                        pu = ps.tile([P, 512], f32, tag="pmm")
                        for kt in range(DT):
                            nc.tensor.matmul(pu[:, :cn], wtile(u1t, kt, dt),
                                             hsl(hin, kt, c0, cn),
                                             start=(kt == 0), stop=False)
                        for kt in range(DT):
                            nc.tensor.matmul(pu[:, :cn], wtile(u1b, kt, dt),
                                             agg[:, kt * T + c0: kt * T + c0 + cn],
                                             start=False, stop=(kt == DT - 1))
                        nc.scalar.activation(G[:, dt * T + c0: dt * T + c0 + cn],
                                             pu[:, :cn], AF.Gelu,
                                             bias=ub1[:, dt: dt + 1])

                    # ---- stage 5: V = u2.T@G (+ h via identity mm) ; x^2
                    for dt in range(DT):
                        pv = ps.tile([P, 512], f32, tag="pmm")
                        for kt in range(DT):
                            nc.tensor.matmul(pv[:, :cn], wtile(u2, kt, dt),
                                             G[:, kt * T + c0: kt * T + c0 + cn],
                                             start=(kt == 0), stop=False)
                        nc.tensor.matmul(pv[:, :cn], iden[:],
                                         hsl(hin, dt, c0, cn),
                                         start=False, stop=True)
                        nc.scalar.activation(hsl(hout, dt, c0, cn), pv[:, :cn],
                                             AF.Identity, bias=ub2[:, dt: dt + 1])
                        nc.gpsimd.tensor_tensor(
                            x2[:, dt * T + c0: dt * T + c0 + cn],
                            hsl(hout, dt, c0, cn), hsl(hout, dt, c0, cn),
                            ALU.mult)

                    # ---- stage 6: LN
                    pr0 = psr.tile([1, 512], f32, tag="prow")
                    pr1 = psr.tile([1, 512], f32, tag="prow")
                    for kt in range(DT):
                        nc.tensor.matmul(pr0[:, :cn], ones_col[:],
                                         hsl(hout, kt, c0, cn),
                                         start=(kt == 0), stop=(kt == DT - 1))
                    for kt in range(DT):
                        nc.tensor.matmul(pr1[:, :cn], ones_col[:],
                                         x2[:, kt * T + c0: kt * T + c0 + cn],
                                         start=(kt == 0), stop=(kt == DT - 1))
                    nmu = rowAB[:, c0: c0 + cn]
                    rst = rowAB[:, T + c0: T + c0 + cn]
                    t0 = rowCD[:, c0: c0 + cn]
                    nc.vector.tensor_scalar_mul(nmu, pr0[:, :cn], -1.0 / D)
                    nc.vector.tensor_tensor(t0, nmu, nmu, ALU.mult)   # mu^2
                    nc.vector.scalar_tensor_tensor(t0, pr1[:, :cn], 1.0 / D, t0,
                                                   ALU.mult, ALU.subtract)
                    nc.scalar.activation(t0, t0, AF.Ln, bias=EPS)
                    nc.scalar.activation(rst, t0, AF.Exp, scale=-0.5)
                    nc.vector.tensor_tensor(nmu, nmu, rst, ALU.mult)
                    pb0 = ps.tile([P, 512], f32, tag="pmm")
                    pb1 = ps.tile([P, 512], f32, tag="pmm")
                    nc.tensor.matmul(pb0[:, :cn], ones_row[:], nmu,
                                     start=True, stop=True)
                    nc.tensor.matmul(pb1[:, :cn], ones_row[:], rst,
                                     start=True, stop=True)
                    bc0 = wp.tile([P, CN], f32, tag="bc0")
                    nc.vector.tensor_copy(bc0[:, :cn], pb0[:, :cn])
                    for dt in range(DT):
                        xs = xn[:, dt * T + c0: dt * T + c0 + cn]
                        nc.vector.tensor_tensor(xs, hsl(hout, dt, c0, cn),
                                                pb1[:, :cn], ALU.mult)
                        nc.gpsimd.tensor_tensor(xs, xs, bc0[:, :cn], ALU.add)
                        nc.vector.tensor_scalar(hsl(hout, dt, c0, cn), xs,
                                                lng[:, dt: dt + 1],
                                                lnb[:, dt: dt + 1],
                                                ALU.mult, ALU.add)

                    # zero pad margin on sequence-start cores (chunk 0)
                    if ci == 0 and r < N_ROUNDS - 1:
                        for dt in range(DT):
                            nc.gpsimd.tensor_tensor(
                                hsl(hout, dt, 0, HALO), hsl(hout, dt, 0, HALO),
                                hmask[:], ALU.mult)

            hfin = hbufs[N_ROUNDS % 2]
            qs = [nc.sync, nc.gpsimd]
            for dt in range(DT):
                for ci, (c0, cn) in enumerate(CHUNKS):
                    lo = max(c0, HALO)
                    hi = c0 + cn
                    qs[(dt + ci) % 2].dma_start(
                        out=out_d[dt, :, lo - HALO: hi - HALO],
                        in_=hsl(hfin, dt, lo, hi - lo).bitcast(f32))

    nc.finalize()
    return nc


_NC_CACHE = {}


def _get_nc():
    if "nc" not in _NC_CACHE:
        _NC_CACHE["nc"] = build_nc()
    return _NC_CACHE["nc"]


def _prep_inputs(chunk_summaries, msg_w1, msg_b1, msg_w2, msg_b2,
                 upd_w1, upd_b1, upd_w2, upd_b2, ln_g, ln_b):
    h = np.asarray(chunk_summaries, np.float32)          # (B, N, D)
    w1 = np.asarray(msg_w1, np.float32)                  # (2D, D)
    w2 = np.asarray(msg_w2, np.float32)                  # (D, D)
    u1 = np.asarray(upd_w1, np.float32)
    u2 = np.asarray(upd_w2, np.float32)

    def pack_w(w, dt_np):
        return np.ascontiguousarray(w.reshape(DT, P, D).astype(dt_np))

    def pack_b2(b):
        return np.ascontiguousarray(np.asarray(b, np.float32).reshape(DT, P).T)

    common = {
        "w1t": pack_w(w1[:D], np.float32),
        "w1b": pack_w(w1[D:], np.float32),
        "u1t": pack_w(u1[:D], np.float32),
        "u1b": pack_w(u1[D:], ml_dtypes.bfloat16),
        "w2s": pack_w(w2 / 9.0, ml_dtypes.bfloat16),
        "u2": pack_w(u2, ml_dtypes.bfloat16),
        "b1": pack_b2(msg_b1),
        "b2": pack_b2(msg_b2),
        "ub1": pack_b2(upd_b1),
        "ub2": pack_b2(upd_b2),
        "lng": pack_b2(ln_g),
        "lnb": pack_b2(ln_b),
        "iden": np.eye(P, dtype=np.float32),
    }

    i8 = np.arange(W, dtype=np.float32)
    ea_edge = np.broadcast_to((W - i8), (P, W)).astype(ml_dtypes.bfloat16)
    es_edge = np.broadcast_to((9.0 / (i8 + 1.0)), (P, W)).astype(ml_dtypes.bfloat16)
    ea_mid = np.zeros((P, W), ml_dtypes.bfloat16)
    es_mid = np.ones((P, W), ml_dtypes.bfloat16)
    hm_edge = np.zeros((P, HALO), np.float32)
    hm_mid = np.ones((P, HALO), np.float32)

    in_maps = []
    for core in range(NCORES):
        b = core // 4
        q = core % 4
        n0 = q * NLOC
        if q == 0:
            loc = np.zeros((T, D), np.float32)
            loc[HALO:] = h[b, :NLOC]
            ea, es, hm = ea_edge, es_edge, hm_edge
        else:
            loc = h[b, n0 - HALO: n0 + NLOC]
            ea, es, hm = ea_mid, es_mid, hm_mid
        hloc = np.ascontiguousarray(loc.T.reshape(DT, P, T))
        m = dict(common)
        m["h_in"] = hloc
        m["edge_a"] = ea
        m["edge_s"] = es
        m["hmask"] = hm
        in_maps.append(m)
    return in_maps


def kernel(**inputs) -> np.ndarray:
    nc = _get_nc()
    in_maps = _prep_inputs(**inputs)
    res = run_bass_kernel_spmd(nc, in_maps, list(range(NCORES)))
    out = np.empty((B, N, D), np.float32)
    for core in range(NCORES):
        b = core // 4
        q = core % 4
        o = res.results[core]["out"]          # (DT, P, NLOC)
        out[b, q * NLOC:(q + 1) * NLOC] = o.reshape(D, NLOC).T
    return out


# revision 17
# speedup vs baseline: 1.1001x; 1.1001x over previous
"""Trainium2 Bass kernel for ChunkMessagePassing (gnn_message_passing).

Problem: B=2, N=4096, D=512, 3 rounds of causal windowed (W=8) message
passing. Per round:
    A = h @ w1_top + b1 ; Bv = h @ w1_bot          (first MLP layer, factored)
    S[i] = sum_{k=0..8, valid} gelu(A[i] + Bv[i-k])
    agg[i] = (S[i]/cnt[i]) @ w2 + b2               (sum commutes with linear)
    new_h = h + MLP_upd([h, agg]) ; h = LN(new_h)

Sharding: 8 cores = B(2) x N-quarters(4). Each core gets 1024 tokens plus a
24-token left halo (3 rounds x window 8), computed redundantly. Zero
cross-core communication. Cores at a sequence start get a zero pad instead
of a halo plus a data-driven edge fixup (invalid window taps excluded,
window count < 9) so all 8 cores run one SPMD program.

Layout: D on partitions (4 tiles of 128), tokens on the free axis.
Matmuls in fp32r (1 cyc/row at free>=256, ~1e-4 rel err). Window stage in
bf16 for DVE 2x mode; shifted reads stay 4B-aligned via an odd-shifted Bv
copy. LN stats via ones-matmul over partitions, broadcast back via K=1
matmul. The round body is chunk-pipelined: every stage runs per 262-token
chunk so PE matmul work for chunk c+1 overlaps the DVE/ACT window work of
chunk c.
"""

import numpy as np
import ml_dtypes

import concourse.bacc as bacc
import concourse.mybir as mybir
from concourse.tile import TileContext
from concourse.bass_utils import run_bass_kernel_spmd

f32 = mybir.dt.float32
f32r = mybir.dt.float32r
bf16 = mybir.dt.bfloat16
AF = mybir.ActivationFunctionType
ALU = mybir.AluOpType

B, N, D = 2, 4096, 512
N_ROUNDS = 3
W = 8
W9 = W + 1
NCORES = 8
NLOC = N // 4            # tokens owned per core
HALO = N_ROUNDS * W      # 24
T = NLOC + HALO          # 1048 local tokens incl. halo
DT = 4                   # number of 128-partition d tiles
P = 128
MARG = 8                 # zero margin on the left of Bv buffers
CN = 352                 # max chunk width (>=256 keeps fp32r at 1 cyc/row)
CHUNKS = [(0, 352), (352, 352), (704, 344)]
EPS = 1e-5


def build_nc():
    nc = bacc.Bacc("TRN2")

    # ---- DRAM I/O (per-core data supplied via in_maps) ----
    h_in = nc.dram_tensor("h_in", [DT, P, T], f32r, kind="ExternalInput")
    w1t_d = nc.dram_tensor("w1t", [DT, P, D], f32r, kind="ExternalInput")
    w1b_d = nc.dram_tensor("w1b", [DT, P, D], f32r, kind="ExternalInput")
    u1t_d = nc.dram_tensor("u1t", [DT, P, D], f32r, kind="ExternalInput")
    u1b_d = nc.dram_tensor("u1b", [DT, P, D], bf16, kind="ExternalInput")
    w2s_d = nc.dram_tensor("w2s", [DT, P, D], bf16, kind="ExternalInput")
    u2_d = nc.dram_tensor("u2", [DT, P, D], bf16, kind="ExternalInput")
    b1_d = nc.dram_tensor("b1", [P, DT], f32, kind="ExternalInput")
    b2_d = nc.dram_tensor("b2", [P, DT], f32, kind="ExternalInput")
    ub1_d = nc.dram_tensor("ub1", [P, DT], f32, kind="ExternalInput")
    ub2_d = nc.dram_tensor("ub2", [P, DT], f32, kind="ExternalInput")
    lng_d = nc.dram_tensor("lng", [P, DT], f32, kind="ExternalInput")
    lnb_d = nc.dram_tensor("lnb", [P, DT], f32, kind="ExternalInput")
    iden_d = nc.dram_tensor("iden", [P, P], f32r, kind="ExternalInput")
    ea_d = nc.dram_tensor("edge_a", [P, W], bf16, kind="ExternalInput")
    es_d = nc.dram_tensor("edge_s", [P, W], bf16, kind="ExternalInput")
    hm_d = nc.dram_tensor("hmask", [P, HALO], f32, kind="ExternalInput")
    out_d = nc.dram_tensor("out", [DT, P, NLOC], f32, kind="ExternalOutput")

    with nc.allow_low_precision("bf16/f32r compute validated against reference"), \
            TileContext(nc) as tc:
        with (
            tc.tile_pool(name="const", bufs=1) as cp,
            tc.tile_pool(name="acts", bufs=1) as ap,
            tc.tile_pool(name="wsc", bufs=3) as wp,
            tc.tile_pool(name="psab", bufs=3, space="PSUM") as psab,
            tc.tile_pool(name="ps", bufs=3, space="PSUM") as ps,
            tc.tile_pool(name="psr", bufs=2, space="PSUM") as psr,
        ):
            # ---- constants into SBUF ----
            w1t = cp.tile([P, DT * D], f32r, tag="w1t")
            w1b = cp.tile([P, DT * D], f32r, tag="w1b")
            u1t = cp.tile([P, DT * D], f32r, tag="u1t")
            u1b = cp.tile([P, DT * D], bf16, tag="u1b")
            w2s = cp.tile([P, DT * D], bf16, tag="w2s")
            u2 = cp.tile([P, DT * D], bf16, tag="u2")
            iden = cp.tile([P, P], f32r, tag="iden")
            for t_sb, t_d in ((w1t, w1t_d), (w1b, w1b_d), (u1t, u1t_d),
                              (u1b, u1b_d), (w2s, w2s_d), (u2, u2_d)):
                nc.sync.dma_start(
                    out=t_sb[:].rearrange("p (k d) -> p k d", k=DT),
                    in_=t_d.rearrange("k p d -> p k d"))
            nc.sync.dma_start(out=iden[:], in_=iden_d[:])
            b1 = cp.tile([P, DT], f32, tag="b1")
            b2 = cp.tile([P, DT], f32, tag="b2")
            ub1 = cp.tile([P, DT], f32, tag="ub1")
            ub2 = cp.tile([P, DT], f32, tag="ub2")
            lng = cp.tile([P, DT], f32, tag="lng")
            lnb = cp.tile([P, DT], f32, tag="lnb")
            edge_a = cp.tile([P, W], bf16, tag="edge_a")
            edge_s = cp.tile([P, W], bf16, tag="edge_s")
            hmask = cp.tile([P, HALO], f32, tag="hmask")
            for t_sb, t_d in ((b1, b1_d), (b2, b2_d), (ub1, ub1_d), (ub2, ub2_d),
                              (lng, lng_d), (lnb, lnb_d), (edge_a, ea_d),
                              (edge_s, es_d), (hmask, hm_d)):
                nc.scalar.dma_start(out=t_sb[:], in_=t_d[:])

            ones_sq = cp.tile([P, P], f32r, tag="ones_sq")     # stats lhsT (bcast out)
            ones_f = cp.tile([P, 1], f32, tag="ones_f")
            nc.vector.memset(ones_f[:], 1.0)
            nc.vector.tensor_copy(ones_sq[:], ones_f[:].to_broadcast([P, P]))
            czero = cp.tile([P, 1], f32, tag="czero")
            ceps = cp.tile([P, 1], f32, tag="ceps")
            nc.vector.memset(czero[:], 0.0)
            nc.vector.memset(ceps[:], EPS)
            nc.const_aps.aps[(f32, 0.0)] = czero[:]
            nc.const_aps.aps[(f32, EPS)] = ceps[:]

            # ---- activations (persistent, reused across rounds) ----
            h0 = ap.tile([P, DT * T], f32r, tag="h0")
            h1 = ap.tile([P, DT * T], f32r, tag="h1")
            A = ap.tile([P, DT * T], bf16, tag="A")
            BVW = MARG + T + 2
            Bv = ap.tile([P, DT * BVW], bf16, tag="Bv")
            Bvo = ap.tile([P, DT * BVW], bf16, tag="Bvo")
            S = ap.tile([P, DT * T], bf16, tag="S")
            agg = ap.tile([P, DT * T], bf16, tag="agg")
            x2 = ap.tile([P, DT * T], f32r, tag="x2")
            rowAB = ap.tile([1, 2 * T], f32r, tag="rowAB")   # [-mu*rstd | rstd]
            rowCD = ap.tile([1, 2 * T], f32, tag="rowCD")
            ga8 = ap.tile([P, W], bf16, tag="ga8")
            xn = x2        # aliases: x2[*,c] dead (stats read) before xn[*,c]
            G = A          # G written after A's last read each round

            for dt in range(DT):
                nc.vector.memset(Bv[:, dt * BVW: dt * BVW + MARG], 0.0)
                nc.vector.memset(Bvo[:, dt * BVW: dt * BVW + MARG + 1], 0.0)

            # round-1 input, chunked so stage-1 starts on the first chunk;
            # separate queue (gpsimd) so it overlaps the weight DMAs
            for (c0, cn) in CHUNKS:
                for dt in range(DT):
                    nc.gpsimd.dma_start(out=h0[:, dt * T + c0: dt * T + c0 + cn],
                                        in_=h_in[dt, :, c0: c0 + cn])

            def hsl(h, dt, c0, n):
                return h[:, dt * T + c0: dt * T + c0 + n]

            def wtile(w, kt, dt):
                return w[:, kt * D + dt * P: kt * D + dt * P + P]

            hbufs = [h0, h1]
            for r in range(N_ROUNDS):
                hin = hbufs[r % 2]
                hout = hbufs[(r + 1) % 2]

                for ci, (c0, cn) in enumerate(CHUNKS):
                    # ---- stage 1: A / Bv matmuls for this chunk
                    for dt in range(DT):
                        pa = psab.tile([P, 512], f32, tag="pab")
                        for kt in range(DT):
                            nc.tensor.matmul(pa[:, :cn], wtile(w1t, kt, dt),
                                             hsl(hin, kt, c0, cn),
                                             start=(kt == 0), stop=(kt == DT - 1))
                        nc.scalar.activation(A[:, dt * T + c0: dt * T + c0 + cn],
                                             pa[:, :cn], AF.Copy)
                        pb = psab.tile([P, 512], f32, tag="pab")
                        for kt in range(DT):
                            nc.tensor.matmul(pb[:, :cn], wtile(w1b, kt, dt),
                                             hsl(hin, kt, c0, cn),
                                             start=(kt == 0), stop=(kt == DT - 1))
                        base = dt * BVW + MARG + c0
                        nc.scalar.activation(Bv[:, base: base + cn], pb[:, :cn],
                                             AF.Copy)
                        nc.vector.tensor_copy(Bvo[:, base + 1: base + 1 + cn],
                                              Bv[:, base: base + cn])

                    # ---- stage 2: windowed gelu-sum -> S (this chunk)
                    for dt in range(DT):
                        tmp = wp.tile([P, W9 * CN], bf16, tag="tmp")
                        g = wp.tile([P, W9 * CN], bf16, tag="g")
                        a_sl = A[:, dt * T + c0: dt * T + c0 + cn]
                        for k in range(W9):
                            if k % 2 == 0:
                                bsl = Bv[:, dt * BVW + MARG - k + c0:
                                         dt * BVW + MARG - k + c0 + cn]
                            else:
                                bsl = Bvo[:, dt * BVW + MARG + 1 - k + c0:
                                          dt * BVW + MARG + 1 - k + c0 + cn]
                            nc.vector.tensor_tensor(
                                tmp[:, k * cn:(k + 1) * cn], a_sl, bsl, ALU.add)
                        nc.scalar.activation(g[:, : W9 * cn], tmp[:, : W9 * cn],
                                             AF.Gelu, bias=b1[:, dt: dt + 1])
                        nc.vector.tensor_tensor(tmp[:, 0: 4 * cn], g[:, 0: 4 * cn],
                                                g[:, 4 * cn: 8 * cn], ALU.add)
                        nc.vector.tensor_tensor(tmp[:, 0: 2 * cn], tmp[:, 0: 2 * cn],
                                                tmp[:, 2 * cn: 4 * cn], ALU.add)
                        nc.vector.tensor_tensor(tmp[:, 0: cn], tmp[:, 0: cn],
                                                tmp[:, cn: 2 * cn], ALU.add)
                        nc.vector.tensor_tensor(
                            S[:, dt * T + c0: dt * T + c0 + cn],
                            tmp[:, 0: cn], g[:, 8 * cn: 9 * cn], ALU.add)

                    # ---- edge fixup (chunk 0 only; no-op off sequence starts)
                    if ci == 0:
                        for dt in range(DT):
                            sle = S[:, dt * T + HALO: dt * T + HALO + W]
                            nc.scalar.activation(
                                ga8[:], A[:, dt * T + HALO: dt * T + HALO + W],
                                AF.Gelu, bias=b1[:, dt: dt + 1])
                            nc.vector.tensor_tensor(ga8[:], ga8[:], edge_a[:],
                                                    ALU.mult)
                            nc.vector.tensor_tensor(sle, sle, ga8[:], ALU.subtract)
                            nc.vector.tensor_tensor(sle, sle, edge_s[:], ALU.mult)

                    # ---- stage 3: agg = S @ w2s + b2
                    for dt in range(DT):
                        pg = ps.tile([P, 512], f32, tag="pmm")
                        for kt in range(DT):
                            nc.tensor.matmul(pg[:, :cn], wtile(w2s, kt, dt),
                                             S[:, kt * T + c0: kt * T + c0 + cn],
                                             start=(kt == 0), stop=(kt == DT - 1))
                        nc.scalar.activation(agg[:, dt * T + c0: dt * T + c0 + cn],
                                             pg[:, :cn], AF.Identity,
                                             bias=b2[:, dt: dt + 1])

                    # ---- stage 4: U = u1t.T@h + u1b.T@agg ; G = gelu(U+ub1)
                    for dt in range(DT):
                        pu = ps.tile([P, 512], f32, tag="pmm")
                        for kt in range(DT):
                            nc.tensor.matmul(pu[:, :cn], wtile(u1t, kt, dt),
                                             hsl(hin, kt, c0, cn),
                                             start=(kt == 0), stop=False)
                        for kt in range(DT):
                            nc.tensor.matmul(pu[:, :cn], wtile(u1b, kt, dt),
                                             agg[:, kt * T + c0: kt * T + c0 + cn],
                                             start=False, stop=(kt == DT - 1))
                        nc.scalar.activation(G[:, dt * T + c0: dt * T + c0 + cn],
                                             pu[:, :cn], AF.Gelu,
                                             bias=ub1[:, dt: dt + 1])

                    # ---- stage 5: V = u2.T@G (+ h via identity mm) ; x^2
                    for dt in range(DT):
                        pv = ps.tile([P, 512], f32, tag="pmm")
                        for kt in range(DT):
                            nc.tensor.matmul(pv[:, :cn], wtile(u2, kt, dt),
                                             G[:, kt * T + c0: kt * T + c0 + cn],
                                             start=(kt == 0), stop=False)
                        nc.tensor.matmul(pv[:, :cn], iden[:],
                                         hsl(hin, dt, c0, cn),
                                         start=False, stop=True)
                        nc.scalar.activation(hsl(hout, dt, c0, cn), pv[:, :cn],
                                             AF.Identity, bias=ub2[:, dt: dt + 1])
                        nc.gpsimd.tensor_tensor(
                            x2[:, dt * T + c0: dt * T + c0 + cn],
                            hsl(hout, dt, c0, cn), hsl(hout, dt, c0, cn),
                            ALU.mult)

                    # ---- stage 6: LN via broadcast ones-matmul stats
                    # lhsT = all-ones (128,128): every out partition gets the
                    # D-sum, so stats arrive pre-broadcast (same PE cycles).
                    pr0 = psr.tile([P, 512], f32, tag="prow")
                    pr1 = psr.tile([P, 512], f32, tag="prow")
                    for kt in range(DT):
                        nc.tensor.matmul(pr0[:, :cn], ones_sq[:],
                                         hsl(hout, kt, c0, cn),
                                         start=(kt == 0), stop=(kt == DT - 1))
                    for kt in range(DT):
                        nc.tensor.matmul(pr1[:, :cn], ones_sq[:],
                                         x2[:, kt * T + c0: kt * T + c0 + cn],
                                         start=(kt == 0), stop=(kt == DT - 1))
                    nmu_b = wp.tile([P, CN], f32, tag="nmu_b")
                    rst_b = wp.tile([P, CN], f32, tag="rst_b")
                    c0_b = wp.tile([P, CN], f32, tag="c0_b")
                    nc.vector.tensor_scalar_mul(nmu_b[:, :cn], pr0[:, :cn], -1.0 / D)
                    nc.vector.tensor_tensor(c0_b[:, :cn], nmu_b[:, :cn],
                                            nmu_b[:, :cn], ALU.mult)   # mu^2
                    nc.vector.scalar_tensor_tensor(rst_b[:, :cn], pr1[:, :cn],
                                                   1.0 / D, c0_b[:, :cn],
                                                   ALU.mult, ALU.subtract)  # var
                    nc.scalar.activation(rst_b[:, :cn], rst_b[:, :cn], AF.Ln,
                                         bias=EPS)
                    nc.scalar.activation(rst_b[:, :cn], rst_b[:, :cn], AF.Exp,
                                         scale=-0.5)
                    nc.vector.tensor_tensor(c0_b[:, :cn], nmu_b[:, :cn],
                                            rst_b[:, :cn], ALU.mult)
                    for dt in range(DT):
                        xs = xn[:, dt * T + c0: dt * T + c0 + cn]
                        nc.vector.tensor_tensor(xs, hsl(hout, dt, c0, cn),
                                                rst_b[:, :cn], ALU.mult)
                        nc.gpsimd.tensor_tensor(xs, xs, c0_b[:, :cn], ALU.add)
                        nc.vector.tensor_scalar(hsl(hout, dt, c0, cn), xs,
                                                lng[:, dt: dt + 1],
                                                lnb[:, dt: dt + 1],
                                                ALU.mult, ALU.add)

                    # zero pad margin on sequence-start cores (chunk 0)
                    if ci == 0 and r < N_ROUNDS - 1:
                        for dt in range(DT):
                            nc.gpsimd.tensor_tensor(
                                hsl(hout, dt, 0, HALO), hsl(hout, dt, 0, HALO),
                                hmask[:], ALU.mult)

            hfin = hbufs[N_ROUNDS % 2]
            qs = [nc.sync, nc.gpsimd]
            for dt in range(DT):
                for ci, (c0, cn) in enumerate(CHUNKS):
                    lo = max(c0, HALO)
                    hi = c0 + cn
                    qs[(dt + ci) % 2].dma_start(
                        out=out_d[dt, :, lo - HALO: hi - HALO],
                        in_=hsl(hfin, dt, lo, hi - lo).bitcast(f32))

    nc.finalize()
    return nc


_NC_CACHE = {}


def _get_nc():
    if "nc" not in _NC_CACHE:
        _NC_CACHE["nc"] = build_nc()
    return _NC_CACHE["nc"]


def _prep_inputs(chunk_summaries, msg_w1, msg_b1, msg_w2, msg_b2,
                 upd_w1, upd_b1, upd_w2, upd_b2, ln_g, ln_b):
    h = np.asarray(chunk_summaries, np.float32)          # (B, N, D)
    w1 = np.asarray(msg_w1, np.float32)                  # (2D, D)
    w2 = np.asarray(msg_w2, np.float32)                  # (D, D)
    u1 = np.asarray(upd_w1, np.float32)
    u2 = np.asarray(upd_w2, np.float32)

    def pack_w(w, dt_np):
        return np.ascontiguousarray(w.reshape(DT, P, D).astype(dt_np))

    def pack_b2(b):
        return np.ascontiguousarray(np.asarray(b, np.float32).reshape(DT, P).T)

    common = {
        "w1t": pack_w(w1[:D], np.float32),
        "w1b": pack_w(w1[D:], np.float32),
        "u1t": pack_w(u1[:D], np.float32),
        "u1b": pack_w(u1[D:], ml_dtypes.bfloat16),
        "w2s": pack_w(w2 / 9.0, ml_dtypes.bfloat16),
        "u2": pack_w(u2, ml_dtypes.bfloat16),
        "b1": pack_b2(msg_b1),
        "b2": pack_b2(msg_b2),
        "ub1": pack_b2(upd_b1),
        "ub2": pack_b2(upd_b2),
        "lng": pack_b2(ln_g),
        "lnb": pack_b2(ln_b),
        "iden": np.eye(P, dtype=np.float32),
    }

    i8 = np.arange(W, dtype=np.float32)
    ea_edge = np.broadcast_to((W - i8), (P, W)).astype(ml_dtypes.bfloat16)
    es_edge = np.broadcast_to((9.0 / (i8 + 1.0)), (P, W)).astype(ml_dtypes.bfloat16)
    ea_mid = np.zeros((P, W), ml_dtypes.bfloat16)
    es_mid = np.ones((P, W), ml_dtypes.bfloat16)
    hm_edge = np.zeros((P, HALO), np.float32)
    hm_mid = np.ones((P, HALO), np.float32)

    in_maps = []
    for core in range(NCORES):
        b = core // 4
        q = core % 4
        n0 = q * NLOC
        if q == 0:
            loc = np.zeros((T, D), np.float32)
            loc[HALO:] = h[b, :NLOC]
            ea, es, hm = ea_edge, es_edge, hm_edge
        else:
            loc = h[b, n0 - HALO: n0 + NLOC]
            ea, es, hm = ea_mid, es_mid, hm_mid
        hloc = np.ascontiguousarray(loc.T.reshape(DT, P, T))
        m = dict(common)
        m["h_in"] = hloc
        m["edge_a"] = ea
        m["edge_s"] = es
        m["hmask"] = hm
        in_maps.append(m)
    return in_maps


def kernel(**inputs) -> np.ndarray:
    nc = _get_nc()
    in_maps = _prep_inputs(**inputs)
    res = run_bass_kernel_spmd(nc, in_maps, list(range(NCORES)))
    out = np.empty((B, N, D), np.float32)
    for core in range(NCORES):
        b = core // 4
        q = core % 4
        o = res.results[core]["out"]          # (DT, P, NLOC)
        out[b, q * NLOC:(q + 1) * NLOC] = o.reshape(D, NLOC).T
    return out
